# revision 13
# baseline (speedup 1.0000x reference)
"""DeepSeek-style dense MLP (dequant + silu-gated) on 8 TRN2 NeuronCores.

Strategy: data-parallel over the 8192 tokens (1024/core). Host folds the
per-128x128-block dequant scales into the weights (exact fp32 multiply, same
as the reference) and pre-transposes everything into PE-friendly layouts.

Per core (all matmuls fp32r = full PE rate, fp32 PSUM accumulation):
  phase A: gate/up = w0t.T @ xT, h = silu(gate)*up kept resident in SBUF
           ([128 part, 44 iblk, 512 tok] per 512-token chunk)
  phase B: out[d, t] = sum_i w2t[i].T @ h[i]  (contraction over inter dim)

Layouts (per core):
  xT  [16, 128, 1024]   xT[k,p,t] = x[t0+t, k*128+p]
  w0t [44, 128, 16, 128] w0t[i,p,k,c] = dequant(w0)[i*128+c, k*128+p]
  w1t same as w0t
  w2t [16, 128, 44, 128] w2t[m,p,i,c] = dequant(w2)[m*128+c, i*128+p]
  out [16, 128, 1024]    out[m,p,t] = y[t0+t, m*128+p]
"""

import time

import numpy as np

import concourse.bass as bass
import concourse.mybir as mybir
import concourse.tile as tile
from concourse import bacc

P = 128
D_MODEL = 2048
INTER = 5632
TOKENS = 8192
NCORES = 8
TS = TOKENS // NCORES          # 1024 tokens per core
TCH = 512                      # token chunk (psum free dim / fp32 moving max)
NT = TS // TCH                 # 2 chunks
KD = D_MODEL // P              # 16 contraction subtiles (phase A)
NI = INTER // P                # 44 inter blocks
ND = D_MODEL // P              # 16 output d blocks
BLOCK = 128

F32 = mybir.dt.float32
F32R = mybir.dt.float32r
AF = mybir.ActivationFunctionType

_CACHE = {}


def _build_nc(repeat=1):
    nc = bacc.Bacc(None, target_bir_lowering=False)
    xT = nc.declare_dram_parameter("xT", [KD, P, TS], F32, isOutput=False)
    w0t = nc.declare_dram_parameter("w0t", [NI, P, KD, P], F32, isOutput=False)
    w1t = nc.declare_dram_parameter("w1t", [NI, P, KD, P], F32, isOutput=False)
    w2t = nc.declare_dram_parameter("w2t", [ND, P, NI, P], F32, isOutput=False)
    out = nc.declare_dram_parameter("out", [ND, P, TS], F32, isOutput=True)

    with tile.TileContext(nc) as tc:
        with tc.tile_pool(name="hpool", bufs=1) as hpool, \
             tc.tile_pool(name="xpool", bufs=1) as xpool, \
             tc.tile_pool(name="wpool", bufs=2) as wpool, \
             tc.tile_pool(name="w2pool", bufs=2) as w2pool, \
             tc.tile_pool(name="evpool", bufs=2) as evpool, \
             tc.tile_pool(name="opool", bufs=2) as opool, \
             tc.tile_pool(name="psA", bufs=2, space="PSUM") as psA, \
             tc.tile_pool(name="psB", bufs=2, space="PSUM") as psB:
            h = hpool.tile([P, NI, TCH], F32R)          # 90KB/part, reused per chunk
            for t in range(NT * repeat):
                t = t % NT
                tsl = bass.ds(t * TCH, TCH)
                xt = xpool.tile([P, KD, TCH], F32R, name="xt")   # 32KB/part
                nc.scalar.dma_start(
                    out=xt[:],
                    in_=xT[:, :, tsl].rearrange("k p n -> p k n").bitcast(F32R),
                )
                # ---- phase A: h = silu(w0t.T @ x) * (w1t.T @ x) ----
                for i in range(NI):
                    w0 = wpool.tile([P, KD, P], F32R, name="w0")  # 8KB/part
                    w1 = wpool.tile([P, KD, P], F32R, name="w1")
                    nc.sync.dma_start(out=w0[:], in_=w0t[i].bitcast(F32R))
                    nc.sync.dma_start(out=w1[:], in_=w1t[i].bitcast(F32R))
                    pg = psA.tile([P, TCH], F32, name="pg")
                    pu = psA.tile([P, TCH], F32, name="pu")
                    for k in range(KD):
                        nc.tensor.matmul(pg[:], lhsT=w0[:, k, :], rhs=xt[:, k, :],
                                         start=(k == 0), stop=(k == KD - 1))
                    for k in range(KD):
                        nc.tensor.matmul(pu[:], lhsT=w1[:, k, :], rhs=xt[:, k, :],
                                         start=(k == 0), stop=(k == KD - 1))
                    sg = evpool.tile([P, TCH], F32, name="sg")
                    nc.scalar.activation(sg[:], pg[:], AF.Silu)
                    nc.vector.tensor_mul(h[:, i, :], sg[:], pu[:])
                # ---- phase B: out[m] = sum_i w2t[m,i].T @ h[i] ----
                for m in range(ND):
                    w2 = w2pool.tile([P, NI, P], F32R, name="w2")  # 22.5KB/part
                    nc.gpsimd.dma_start(out=w2[:], in_=w2t[m].bitcast(F32R))
                    po = psB.tile([P, TCH], F32, name="po")
                    for i in range(NI):
                        nc.tensor.matmul(po[:], lhsT=w2[:, i, :], rhs=h[:, i, :],
                                         start=(i == 0), stop=(i == NI - 1))
                    ot = opool.tile([P, TCH], F32, name="ot")
                    nc.scalar.activation(ot[:], po[:], AF.Copy)
                    nc.scalar.dma_start(out=out[m, :, tsl], in_=ot[:])
    nc.compile()
    return nc


def _dequant(w, s):
    m, n = w.shape
    wb = w.reshape(m // BLOCK, BLOCK, n // BLOCK, BLOCK)
    return (wb * s[:, None, :, None]).reshape(m, n)


def _prep_weights(w0, s0, w1, s1, w2, s2):
    # w0t[i,p,k,c] = dq0[i*128+c, k*128+p]
    dq0 = _dequant(w0, s0).reshape(NI, P, KD, P)       # [i, c, k, p]
    w0t = np.ascontiguousarray(dq0.transpose(0, 3, 2, 1))
    dq1 = _dequant(w1, s1).reshape(NI, P, KD, P)
    w1t = np.ascontiguousarray(dq1.transpose(0, 3, 2, 1))
    # w2t[m,p,i,c] = dq2[m*128+c, i*128+p]
    dq2 = _dequant(w2, s2).reshape(ND, P, NI, P)       # [m, c, i, p]
    w2t = np.ascontiguousarray(dq2.transpose(0, 3, 2, 1))
    return w0t, w1t, w2t


def _prep_x(x):
    """x [8192, 2048] -> per-core xT [16, 128, 1024]."""
    shards = []
    for c in range(NCORES):
        xs = x[c * TS:(c + 1) * TS]                    # [1024, 2048]
        shards.append(np.ascontiguousarray(xs.T.reshape(KD, P, TS)))
    return shards


def _get_runner(repeat=1):
    """Build (once per repeat count) a sharded jitted executor over the 8 cores.

    Modeled on concourse.bass2jax.run_bass_via_pjrt, but cached and fed
    device-resident inputs so repeat calls don't re-trace or re-transfer.
    """
    key = ("runner", repeat)
    if key in _CACHE:
        return _CACHE[key]

    import jax
    from jax.experimental.shard_map import shard_map
    from jax.sharding import Mesh, NamedSharding, PartitionSpec

    from concourse import bass2jax

    nc = _build_nc(repeat)
    bass2jax.install_neuronx_cc_hook()

    partition_name = nc.partition_id_tensor.name if nc.partition_id_tensor else None
    in_names, out_names, out_avals = [], [], []
    for alloc in nc.m.functions[0].allocations:
        if not isinstance(alloc, mybir.MemoryLocationSet):
            continue
        name = alloc.memorylocations[0].name
        if alloc.kind == "ExternalInput":
            if name != partition_name:
                in_names.append(name)
        elif alloc.kind == "ExternalOutput":
            out_names.append(name)
            out_avals.append(
                jax.core.ShapedArray(tuple(alloc.tensor_shape), mybir.dt.np(alloc.dtype))
            )
    n_params = len(in_names)
    all_in_names = list(in_names) + list(out_names)
    if partition_name is not None:
        all_in_names.append(partition_name)

    def _body(*args):
        operands = list(args)
        if partition_name is not None:
            operands.append(bass2jax.partition_id_tensor())
        outs = bass2jax._bass_exec_p.bind(
            *operands,
            out_avals=tuple(out_avals),
            in_names=tuple(all_in_names),
            out_names=tuple(out_names),
            lowering_input_output_aliases=(),
            sim_require_finite=True,
            sim_require_nnan=True,
            nc=nc,
        )
        return tuple(outs)

    devices = jax.devices()[:NCORES]
    mesh = Mesh(np.asarray(devices), ("core",))
    spec = PartitionSpec("core")
    fn = jax.jit(
        shard_map(
            _body,
            mesh=mesh,
            in_specs=(spec,) * (n_params + len(out_names)),
            out_specs=(spec,) * len(out_names),
            check_rep=False,
        ),
        keep_unused=True,
    )
    sharding = NamedSharding(mesh, spec)
    runner = {
        "fn": fn,
        "in_names": in_names,
        "out_names": out_names,
        "out_avals": out_avals,
        "sharding": sharding,
        "jax": jax,
    }
    _CACHE[key] = runner
    return runner


def _device_args(inputs):
    """Host-prep + transfer all per-core inputs; returns device arrays."""
    runner = _get_runner()
    jax = runner["jax"]
    x = np.asarray(inputs["x"], dtype=np.float32)
    w0t, w1t, w2t = _prep_weights(
        np.asarray(inputs["w0"], dtype=np.float32),
        np.asarray(inputs["s0"], dtype=np.float32),
        np.asarray(inputs["w1"], dtype=np.float32),
        np.asarray(inputs["s1"], dtype=np.float32),
        np.asarray(inputs["w2"], dtype=np.float32),
        np.asarray(inputs["s2"], dtype=np.float32),
    )
    xs = _prep_x(x)
    per_core = {
        "xT": xs,
        "w0t": [w0t] * NCORES,
        "w1t": [w1t] * NCORES,
        "w2t": [w2t] * NCORES,
    }
    args = []
    for name in runner["in_names"]:
        glob = np.concatenate(per_core[name], axis=0)
        args.append(jax.device_put(glob, runner["sharding"]))
    for aval in runner["out_avals"]:
        shape = (NCORES * aval.shape[0], *aval.shape[1:])
        args.append(jax.device_put(np.zeros(shape, aval.dtype), runner["sharding"]))
    return args


def _run_once(args, repeat=1):
    runner = _get_runner(repeat)
    outs = runner["fn"](*args)
    runner["jax"].block_until_ready(outs)
    return outs


def _assemble(outs):
    out = np.asarray(outs[0])                          # [8*16, 128, 1024]
    out = out.reshape(NCORES, D_MODEL, TS)             # [core, d, t]
    return np.ascontiguousarray(out.transpose(0, 2, 1).reshape(TOKENS, D_MODEL))


def kernel(x, w0, s0, w1, s1, w2, s2):
    args = _device_args(
        {"x": x, "w0": w0, "s0": s0, "w1": w1, "s1": s1, "w2": w2, "s2": s2}
    )
    return _assemble(_run_once(args))


def _batch_wall(args, repeat, iters):
    """Dispatch `iters` executions async, block once; per-call seconds."""
    runner = _get_runner(repeat)
    fn, jax = runner["fn"], runner["jax"]
    jax.block_until_ready(fn(*args))  # warmup / compile
    best = float("inf")
    for _ in range(3):
        t0 = time.perf_counter()
        rs = [fn(*args) for _ in range(iters)]
        jax.block_until_ready(rs)
        t1 = time.perf_counter()
        best = min(best, (t1 - t0) / iters)
    return best


def time_device(inputs, iters=24, hi_repeat=5):
    """Estimate pure device time (ns) by differencing repeat counts.

    Per-call time = dispatch cost + R * kernel_time; async batching makes
    dispatch cost small and stable, and the repeat differential cancels it:
    (percall(R) - percall(1)) / (R - 1) isolates kernel_time.
    """
    args = _device_args(inputs)
    t1 = _batch_wall(args, 1, iters)
    tR = _batch_wall(args, hi_repeat, iters)
    hw = (tR - t1) / (hi_repeat - 1)
    return {"hw_ns": hw * 1e9, "wall1_ns": t1 * 1e9, "wallR_ns": tR * 1e9}


# revision 14
# speedup vs baseline: 1.7851x; 1.7851x over previous
"""DeepSeek-style dense MLP (dequant + silu-gated) on 8 TRN2 NeuronCores.

Strategy: data-parallel over the 8192 tokens (1024/core). Host folds the
per-128x128-block dequant scales into the weights (exact fp32 multiply, same
as the reference) and pre-transposes everything into PE-friendly layouts.

Per core (all matmuls fp32r = full PE rate, fp32 PSUM accumulation):
  phase A: gate/up = w0t.T @ xT, h = silu(gate)*up kept resident in SBUF
           ([128 part, 44 iblk, 512 tok] per 512-token chunk)
  phase B: out[d, t] = sum_i w2t[i].T @ h[i]  (contraction over inter dim)

Layouts (per core):
  xT  [16, 128, 1024]   xT[k,p,t] = x[t0+t, k*128+p]
  w0t [44, 128, 16, 128] w0t[i,p,k,c] = dequant(w0)[i*128+c, k*128+p]
  w1t same as w0t
  w2t [16, 128, 44, 128] w2t[m,p,i,c] = dequant(w2)[m*128+c, i*128+p]
  out [16, 128, 1024]    out[m,p,t] = y[t0+t, m*128+p]
"""

import time

import numpy as np

import concourse.bass as bass
import concourse.mybir as mybir
import concourse.tile as tile
from concourse import bacc

P = 128
D_MODEL = 2048
INTER = 5632
TOKENS = 8192
NCORES = 8
TS = TOKENS // NCORES          # 1024 tokens per core
TCH = 512                      # token chunk (psum free dim / fp32 moving max)
NT = TS // TCH                 # 2 chunks
KD = D_MODEL // P              # 16 contraction subtiles (phase A)
NI = INTER // P                # 44 inter blocks
ND = D_MODEL // P              # 16 output d blocks
BLOCK = 128

F32 = mybir.dt.float32
F32R = mybir.dt.float32r
AF = mybir.ActivationFunctionType

_CACHE = {}


def _build_nc(repeat=1):
    nc = bacc.Bacc(None, target_bir_lowering=False)
    xT = nc.declare_dram_parameter("xT", [KD, P, TS], F32, isOutput=False)
    w0t = nc.declare_dram_parameter("w0t", [NI, P, KD, P], F32, isOutput=False)
    w1t = nc.declare_dram_parameter("w1t", [NI, P, KD, P], F32, isOutput=False)
    w2t = nc.declare_dram_parameter("w2t", [ND, P, NI, P], F32, isOutput=False)
    out = nc.declare_dram_parameter("out", [ND, P, TS], F32, isOutput=True)

    with tile.TileContext(nc) as tc:
        with tc.tile_pool(name="hpool", bufs=1) as hpool, \
             tc.tile_pool(name="xpool", bufs=1) as xpool, \
             tc.tile_pool(name="wpool", bufs=2) as wpool, \
             tc.tile_pool(name="w2pool", bufs=2) as w2pool, \
             tc.tile_pool(name="evpool", bufs=2) as evpool, \
             tc.tile_pool(name="opool", bufs=2) as opool, \
             tc.tile_pool(name="psA", bufs=2, space="PSUM") as psA, \
             tc.tile_pool(name="psB", bufs=2, space="PSUM") as psB:
            h = hpool.tile([P, NI, TCH], F32R)          # 90KB/part, reused per chunk
            for t in range(NT * repeat):
                t = t % NT
                tsl = bass.ds(t * TCH, TCH)
                xt = xpool.tile([P, KD, TCH], F32R, name="xt")   # 32KB/part
                nc.sync.dma_start(
                    out=xt[:],
                    in_=xT[:, :, tsl].rearrange("k p n -> p k n").bitcast(F32R),
                )
                # ---- phase A: h = silu(w0t.T @ x) * (w1t.T @ x) ----
                for i in range(NI):
                    w0 = wpool.tile([P, KD, P], F32R, name="w0")  # 8KB/part
                    w1 = wpool.tile([P, KD, P], F32R, name="w1")
                    nc.sync.dma_start(out=w0[:], in_=w0t[i].bitcast(F32R))
                    nc.sync.dma_start(out=w1[:], in_=w1t[i].bitcast(F32R))
                    pg = psA.tile([P, TCH], F32, name="pg")
                    pu = psA.tile([P, TCH], F32, name="pu")
                    for k in range(KD):
                        nc.tensor.matmul(pg[:], lhsT=w0[:, k, :], rhs=xt[:, k, :],
                                         start=(k == 0), stop=(k == KD - 1))
                    for k in range(KD):
                        nc.tensor.matmul(pu[:], lhsT=w1[:, k, :], rhs=xt[:, k, :],
                                         start=(k == 0), stop=(k == KD - 1))
                    sg = evpool.tile([P, TCH], F32, name="sg")
                    nc.scalar.activation(sg[:], pg[:], AF.Silu)
                    nc.vector.tensor_mul(h[:, i, :], sg[:], pu[:])
                # ---- phase B: out[m] = sum_i w2t[m,i].T @ h[i] ----
                for m in range(ND):
                    w2 = w2pool.tile([P, NI, P], F32R, name="w2")  # 22.5KB/part
                    nc.sync.dma_start(out=w2[:], in_=w2t[m].bitcast(F32R))
                    po = psB.tile([P, TCH], F32, name="po")
                    for i in range(NI):
                        nc.tensor.matmul(po[:], lhsT=w2[:, i, :], rhs=h[:, i, :],
                                         start=(i == 0), stop=(i == NI - 1))
                    ot = opool.tile([P, TCH], F32, name="ot")
                    nc.scalar.activation(ot[:], po[:], AF.Copy)
                    nc.sync.dma_start(out=out[m, :, tsl], in_=ot[:])
    nc.compile()
    return nc


def _dequant(w, s):
    m, n = w.shape
    wb = w.reshape(m // BLOCK, BLOCK, n // BLOCK, BLOCK)
    return (wb * s[:, None, :, None]).reshape(m, n)


def _prep_weights(w0, s0, w1, s1, w2, s2):
    # w0t[i,p,k,c] = dq0[i*128+c, k*128+p]
    dq0 = _dequant(w0, s0).reshape(NI, P, KD, P)       # [i, c, k, p]
    w0t = np.ascontiguousarray(dq0.transpose(0, 3, 2, 1))
    dq1 = _dequant(w1, s1).reshape(NI, P, KD, P)
    w1t = np.ascontiguousarray(dq1.transpose(0, 3, 2, 1))
    # w2t[m,p,i,c] = dq2[m*128+c, i*128+p]
    dq2 = _dequant(w2, s2).reshape(ND, P, NI, P)       # [m, c, i, p]
    w2t = np.ascontiguousarray(dq2.transpose(0, 3, 2, 1))
    return w0t, w1t, w2t


def _prep_x(x):
    """x [8192, 2048] -> per-core xT [16, 128, 1024]."""
    shards = []
    for c in range(NCORES):
        xs = x[c * TS:(c + 1) * TS]                    # [1024, 2048]
        shards.append(np.ascontiguousarray(xs.T.reshape(KD, P, TS)))
    return shards


def _get_runner(repeat=1):
    """Build (once per repeat count) a sharded jitted executor over the 8 cores.

    Modeled on concourse.bass2jax.run_bass_via_pjrt, but cached and fed
    device-resident inputs so repeat calls don't re-trace or re-transfer.
    """
    key = ("runner", repeat)
    if key in _CACHE:
        return _CACHE[key]

    import jax
    from jax.experimental.shard_map import shard_map
    from jax.sharding import Mesh, NamedSharding, PartitionSpec

    from concourse import bass2jax

    nc = _build_nc(repeat)
    bass2jax.install_neuronx_cc_hook()

    partition_name = nc.partition_id_tensor.name if nc.partition_id_tensor else None
    in_names, out_names, out_avals = [], [], []
    for alloc in nc.m.functions[0].allocations:
        if not isinstance(alloc, mybir.MemoryLocationSet):
            continue
        name = alloc.memorylocations[0].name
        if alloc.kind == "ExternalInput":
            if name != partition_name:
                in_names.append(name)
        elif alloc.kind == "ExternalOutput":
            out_names.append(name)
            out_avals.append(
                jax.core.ShapedArray(tuple(alloc.tensor_shape), mybir.dt.np(alloc.dtype))
            )
    n_params = len(in_names)
    all_in_names = list(in_names) + list(out_names)
    if partition_name is not None:
        all_in_names.append(partition_name)

    def _body(*args):
        operands = list(args)
        if partition_name is not None:
            operands.append(bass2jax.partition_id_tensor())
        outs = bass2jax._bass_exec_p.bind(
            *operands,
            out_avals=tuple(out_avals),
            in_names=tuple(all_in_names),
            out_names=tuple(out_names),
            lowering_input_output_aliases=(),
            sim_require_finite=True,
            sim_require_nnan=True,
            nc=nc,
        )
        return tuple(outs)

    devices = jax.devices()[:NCORES]
    mesh = Mesh(np.asarray(devices), ("core",))
    spec = PartitionSpec("core")
    fn = jax.jit(
        shard_map(
            _body,
            mesh=mesh,
            in_specs=(spec,) * (n_params + len(out_names)),
            out_specs=(spec,) * len(out_names),
            check_rep=False,
        ),
        keep_unused=True,
    )
    sharding = NamedSharding(mesh, spec)
    runner = {
        "fn": fn,
        "in_names": in_names,
        "out_names": out_names,
        "out_avals": out_avals,
        "sharding": sharding,
        "jax": jax,
    }
    _CACHE[key] = runner
    return runner


def _device_args(inputs):
    """Host-prep + transfer all per-core inputs; returns device arrays."""
    runner = _get_runner()
    jax = runner["jax"]
    x = np.asarray(inputs["x"], dtype=np.float32)
    w0t, w1t, w2t = _prep_weights(
        np.asarray(inputs["w0"], dtype=np.float32),
        np.asarray(inputs["s0"], dtype=np.float32),
        np.asarray(inputs["w1"], dtype=np.float32),
        np.asarray(inputs["s1"], dtype=np.float32),
        np.asarray(inputs["w2"], dtype=np.float32),
        np.asarray(inputs["s2"], dtype=np.float32),
    )
    xs = _prep_x(x)
    per_core = {
        "xT": xs,
        "w0t": [w0t] * NCORES,
        "w1t": [w1t] * NCORES,
        "w2t": [w2t] * NCORES,
    }
    args = []
    for name in runner["in_names"]:
        glob = np.concatenate(per_core[name], axis=0)
        args.append(jax.device_put(glob, runner["sharding"]))
    for aval in runner["out_avals"]:
        shape = (NCORES * aval.shape[0], *aval.shape[1:])
        args.append(jax.device_put(np.zeros(shape, aval.dtype), runner["sharding"]))
    return args


def _run_once(args, repeat=1):
    runner = _get_runner(repeat)
    outs = runner["fn"](*args)
    runner["jax"].block_until_ready(outs)
    return outs


def _assemble(outs):
    out = np.asarray(outs[0])                          # [8*16, 128, 1024]
    out = out.reshape(NCORES, D_MODEL, TS)             # [core, d, t]
    return np.ascontiguousarray(out.transpose(0, 2, 1).reshape(TOKENS, D_MODEL))


def kernel(x, w0, s0, w1, s1, w2, s2):
    args = _device_args(
        {"x": x, "w0": w0, "s0": s0, "w1": w1, "s1": s1, "w2": w2, "s2": s2}
    )
    return _assemble(_run_once(args))


def _batch_wall(args, repeat, iters):
    """Dispatch `iters` executions async, block once; per-call seconds."""
    runner = _get_runner(repeat)
    fn, jax = runner["fn"], runner["jax"]
    jax.block_until_ready(fn(*args))  # warmup / compile
    best = float("inf")
    for _ in range(3):
        t0 = time.perf_counter()
        rs = [fn(*args) for _ in range(iters)]
        jax.block_until_ready(rs)
        t1 = time.perf_counter()
        best = min(best, (t1 - t0) / iters)
    return best


def time_device(inputs, iters=24, hi_repeat=5):
    """Estimate pure device time (ns) by differencing repeat counts.

    Per-call time = dispatch cost + R * kernel_time; async batching makes
    dispatch cost small and stable, and the repeat differential cancels it:
    (percall(R) - percall(1)) / (R - 1) isolates kernel_time.
    """
    args = _device_args(inputs)
    t1 = _batch_wall(args, 1, iters)
    tR = _batch_wall(args, hi_repeat, iters)
    hw = (tR - t1) / (hi_repeat - 1)
    return {"hw_ns": hw * 1e9, "wall1_ns": t1 * 1e9, "wallR_ns": tR * 1e9}
